# revision 1
# baseline (speedup 1.0000x reference)
"""Criss-cross attention (nn_CC_attention) Trainium2 kernel.

Sharding: pure data parallel over batch B=8 across 8 NeuronCores; the only
cross-core coupling is the global min/max of energy, exchanged via a tiny
AllGather of (max, -min) followed by a local 8-partition max reduce.

Host-side staging (layout/precision only; all model compute is on-device):
  t1b = fp8(tensor1)  as (H, C, W)   -- keys source
  t2h = fp8(tensor2)  as (H, C, W)   -- energy-W stationary + pooled values
  t2t = fp16(tensor2) as (W, C, H)   -- W branch; carries the exact +tensor2
  out is produced as fp16 (H, C, W), host transposes back to (C, H, W) fp32.

Per-core device algorithm:
  phase 1 (stream 8 chunks of 32 channels, 2 groups of 16 per chunk;
           all input DMAs issued from the Sync engine so no compute FIFO
           ever blocks a DMA issue):
    kW[c][h,k] = avg-pool_w(t1)      (DVE reduce + scale->fp16)
    kH[c][w,k] = t1b[c].T @ P        (PE, t1 stationary; P = pool/8)
    eW[w,k] += t2h[c].T @ kW[c]      (PE, fp8 stationary)
    eH[h,k] += t2t[c].T @ kH[c]      (PE, fp16 stationary)
    val4[32r+k,(c,w)] = pool4.T @ t2h  (PE after the energies; pooled values
                                        replicated on 4 partition strips;
                                        psum->fp8 copies all on ACT)
  boundary:
    local (max,-min) -> AllGather(8x2) -> 8-partition max -> global range;
    exp on ACT; sums; A4[32r+k,h] = 0.5*att_H[h,k] (4 strips, bf16)
                      M_WI[w',w]  = 0.0625*att_W[w,w'//8] + I (bf16)
    (0.5 gamma folded into A4/expmat; the +tensor2 residual rides M_WI's I
     against the fp16 t2t copy.)
  phase 2 (per group, output DMA per 2-group chunk of ~1MB on Sync):
    psum[h, 512r block] = A4[strip r].T @ val4[strip r]   (4 concurrent
                          row-tiled K=16 matmuls)
    psum += t2t[c].T @ M_WI  (per c, fp16 stationary; adds out_W + tensor2)
    = 0.5*out_H + 0.5*out_W + tensor2 ;  ACT/DVE copy -> fp16 -> chunk DMA.
PE warm-up bursts (dummy matmuls on a memset tile) run at kernel start and
after the collective so the HAM clock gate is open for both phases.
The warm-up AllGather has no data dependencies at all so its doorbell fires
as soon as the NEFF preamble ends, absorbing the ~45us first-collective
barrier cost while phase 1 streams.
"""

import numpy as np
from contextlib import ExitStack

import ml_dtypes
import concourse.bass as bass
import concourse.tile as tile
from concourse import bacc, bass_isa, mybir

B, C, H, W, POOL = 8, 256, 128, 128, 8
KH, KW = H // POOL, W // POOL  # 16, 16
NCORES = 8
G = 16       # channels per compute group
CHUNK = 32   # channels per DMA chunk (2 groups)
NWARM1 = 40  # dummy matmuls in the kernel-start PE warm-up burst
NWARM2 = 36  # dummy matmuls in the post-collective PE warm-up burst

F32 = mybir.dt.float32
F16 = mybir.dt.float16
BF16 = mybir.dt.bfloat16
F8 = mybir.dt.float8e4
BF_NP = ml_dtypes.bfloat16
F8_NP = ml_dtypes.float8_e4m3


def host_constants():
    pool_m = np.zeros((H, KH), np.float32)
    for k in range(KH):
        pool_m[k * POOL:(k + 1) * POOL, k] = 1.0 / POOL
    expmat = np.zeros((KH, H), np.float32)
    for k in range(KH):
        expmat[k, k * POOL:(k + 1) * POOL] = 0.5 / POOL  # 0.0625
    # pooled-value projector, replicated on the 4 32-partition strips
    pool4 = np.zeros((H, 128), np.float32)
    for r in range(4):
        pool4[:, 32 * r:32 * r + KH] = pool_m
    # att_kh replicator onto the 4 strips, with the 0.5 gamma folded in
    rep4 = np.zeros((KH, 128), np.float32)
    for r in range(4):
        rep4[:, 32 * r:32 * r + KH] = 0.5 * np.eye(KH, dtype=np.float32)
    return {
        "pool16": pool_m.astype(F8_NP),
        "pool4": pool4.astype(F8_NP),
        "rep4": rep4.astype(BF_NP),
        "ident16": np.eye(H, dtype=np.float32).astype(BF_NP),
        "expmat": expmat.astype(BF_NP),
        "eyefull": np.eye(H, dtype=np.float32),
    }


def build(c_total=C, ncores=NCORES):
    assert c_total % CHUNK == 0
    nchunks = c_total // CHUNK
    ngroups = c_total // G
    nc = bacc.Bacc(trn_type="TRN2", target_bir_lowering=False, debug=False,
                   num_devices=ncores)

    t1b = nc.dram_tensor("t1b", [H, c_total, W], F8, kind="ExternalInput").ap()
    t2h = nc.dram_tensor("t2h", [H, c_total, W], F8, kind="ExternalInput").ap()
    t2t = nc.dram_tensor("t2t", [W, c_total, H], F16, kind="ExternalInput").ap()
    pool16 = nc.dram_tensor("pool16", [H, KH], F8, kind="ExternalInput").ap()
    pool4 = nc.dram_tensor("pool4", [H, 128], F8, kind="ExternalInput").ap()
    rep4 = nc.dram_tensor("rep4", [KH, 128], BF16, kind="ExternalInput").ap()
    ident16 = nc.dram_tensor("ident16", [H, W], BF16, kind="ExternalInput").ap()
    expmat = nc.dram_tensor("expmat", [KH, H], BF16, kind="ExternalInput").ap()
    eyefull = nc.dram_tensor("eyefull", [H, W], F32, kind="ExternalInput").ap()
    out = nc.dram_tensor("out", [H, c_total, W], F16, kind="ExternalOutput").ap()

    with tile.TileContext(nc) as tc, ExitStack() as top:
        # ---- constants ----
        cpool = top.enter_context(tc.tile_pool(name="consts", bufs=1))
        c_pool16 = cpool.tile([H, KH], F8, tag="pool16")
        nc.sync.dma_start(c_pool16[:], pool16[:])
        c_pool4 = cpool.tile([H, 128], F8, tag="pool4")
        nc.sync.dma_start(c_pool4[:], pool4[:])
        c_rep4 = cpool.tile([KH, 128], BF16, tag="rep4")
        nc.sync.dma_start(c_rep4[:], rep4[:])
        c_ident = cpool.tile([H, W], BF16, tag="ident16")
        nc.sync.dma_start(c_ident[:], ident16[:])
        c_expmat = cpool.tile([KH, H], BF16, tag="expmat")
        nc.sync.dma_start(c_expmat[:], expmat[:])
        c_eye = cpool.tile([H, W], F32, tag="eyefull")
        nc.sync.dma_start(c_eye[:], eyefull[:])

        # zero tile for PE warm-up (no data deps -> earliest possible issue)
        wtile0 = cpool.tile([H, W], BF16, tag="wtile0")
        nc.vector.memset(wtile0[:], 0.0)

        resqT = top.enter_context(tc.tile_pool(name="resqT", bufs=nchunks))
        vpool = top.enter_context(tc.tile_pool(name="val4", bufs=1))
        val4 = vpool.tile([128, c_total * W], F8, tag="val4")
        t2t_chunks = []

        psb = ExitStack()  # psum pools released before phase 2
        ps_e = psb.enter_context(tc.tile_pool(name="ps_e", bufs=1, space="PSUM"))
        ps_eW = ps_e.tile([W, KW], F32, tag="eW")
        ps_eH = ps_e.tile([H, KH], F32, tag="eH")
        ps_warm = psb.enter_context(tc.tile_pool(name="ps_warm", bufs=1, space="PSUM"))
        ps_w = ps_warm.tile([H, W], F32, tag="warm")

        spool = top.enter_context(tc.tile_pool(name="soft", bufs=1))
        dram = top.enter_context(tc.tile_pool(name="dram", bufs=1, space="DRAM"))

        # PE warm-up burst #1 (memset input only -> scheduled at kernel start)
        for _ in range(NWARM1):
            nc.tensor.matmul(ps_w[:], wtile0[:], wtile0[:], start=True, stop=True)

        # collective warm-up: dummy AllGather with an unwritten input tile, so
        # the doorbell fires immediately and the first-collective barrier cost
        # overlaps phase 1.
        wc_in = dram.tile([1, 8], F32, tag="wc_in")
        wc_out = dram.tile([ncores, 8], F32, tag="wc_out")
        nc.gpsimd.collective_compute(
            "AllGather", mybir.AluOpType.bypass,
            replica_groups=[list(range(ncores))],
            ins=[wc_in.opt()], outs=[wc_out.opt()],
        )

        # ================= phase 1 =================
        with ExitStack() as ph1:
            pin = ph1.enter_context(tc.tile_pool(name="pin", bufs=3))
            pin2 = ph1.enter_context(tc.tile_pool(name="pin2", bufs=3))
            kpool = ph1.enter_context(tc.tile_pool(name="keys", bufs=4))
            ps_kh = ph1.enter_context(tc.tile_pool(name="ps_kh", bufs=2, space="PSUM"))
            ps_v = ph1.enter_context(tc.tile_pool(name="ps_v", bufs=2, space="PSUM"))

            for ch in range(nchunks):
                c0 = ch * CHUNK
                t2g8 = pin2.tile([H, CHUNK * W], F8, tag="t2g8")
                nc.sync.dma_start(t2g8[:].rearrange("p (c w) -> p c w", c=CHUNK),
                                  t2h[:, c0:c0 + CHUNK, :])
                t1g = pin.tile([H, CHUNK * W], F8, tag="t1g")
                nc.sync.dma_start(t1g[:].rearrange("p (c w) -> p c w", c=CHUNK),
                                  t1b[:, c0:c0 + CHUNK, :])
                t2tg = resqT.tile([W, CHUNK * H], F16, tag="t2tg")
                nc.sync.dma_start(t2tg[:].rearrange("p (c h) -> p c h", c=CHUNK),
                                  t2t[:, c0:c0 + CHUNK, :])
                t2t_chunks.append(t2tg)

                for s in range(CHUNK // G):
                    g = ch * (CHUNK // G) + s
                    qoff = s * G * W   # offset into t1g/t2g8 (H-layout)
                    toff = s * G * H   # offset into t2tg (W-layout)

                    # kW[c][h,k] (fp16) : DVE reduce + DVE scale
                    kWr = kpool.tile([H, G * KW], F32, tag="kWr")
                    nc.vector.tensor_reduce(
                        kWr[:].rearrange("p (c k) -> p c k", c=G),
                        t1g[:, qoff:qoff + G * W].rearrange(
                            "p (c k j) -> p c k j", c=G, j=POOL),
                        axis=mybir.AxisListType.X, op=mybir.AluOpType.add)
                    kW = kpool.tile([H, G * KW], F16, tag="kW")
                    nc.vector.tensor_scalar_mul(kW[:], kWr[:], 1.0 / POOL)

                    # kH[c][w,k] (bf16) = t1b[c].T @ pool16
                    ps_kh_t = ps_kh.tile([W, G * KH], F32, tag="ps_kh")
                    for i in range(G):
                        nc.tensor.matmul(ps_kh_t[:, i * KH:(i + 1) * KH],
                                         t1g[:, qoff + i * W:qoff + (i + 1) * W],
                                         c_pool16[:], start=True, stop=True)
                    kH = kpool.tile([W, G * KH], BF16, tag="kH")
                    nc.vector.tensor_copy(kH[:], ps_kh_t[:])

                    first = (g == 0)
                    last = (g == ngroups - 1)
                    for i in range(G):
                        # eW[w,k] += t2h[c].T @ kW[c]  (fp8 stationary)
                        nc.tensor.matmul(ps_eW[:],
                                         t2g8[:, qoff + i * W:qoff + (i + 1) * W],
                                         kW[:, i * KW:(i + 1) * KW],
                                         start=(first and i == 0),
                                         stop=(last and i == G - 1))
                        # eH[h,k] += t2t[c].T @ kH[c]  (fp16 stationary)
                        nc.tensor.matmul(ps_eH[:],
                                         t2tg[:, toff + i * H:toff + (i + 1) * H],
                                         kH[:, i * KH:(i + 1) * KH],
                                         start=(first and i == 0),
                                         stop=(last and i == G - 1))

                # val4: pooled values of t2 on 4 partition strips. Emitted
                # after the energies so the phase-1 critical path is not
                # delayed; doubles as a HAM gap filler. Copies all on ACT
                # (which has no other phase-1 work and issues no DMAs).
                vbase = ch * CHUNK * W
                for q in range(0, CHUNK * W, 512):
                    ps_vt = ps_v.tile([128, 512], F32, tag="ps_v")
                    nc.tensor.matmul(ps_vt[:], c_pool4[:],
                                     t2g8[:, q:q + 512], start=True, stop=True)
                    nc.scalar.copy(val4[:, vbase + q:vbase + q + 512], ps_vt[:])

        # ================= boundary =================
        e_sb = spool.tile([H, 2 * KH], F32, tag="e_sb")
        nc.vector.tensor_copy(e_sb[:, 0:KH], ps_eH[:])
        nc.vector.tensor_copy(e_sb[:, KH:2 * KH], ps_eW[:])

        # local (max, -min) on every partition, then all-partition max
        pack = spool.tile([H, 2], F32, tag="pack")
        nc.vector.tensor_reduce(pack[:, 0:1], e_sb[:], axis=mybir.AxisListType.X,
                                op=mybir.AluOpType.max)
        rmin = spool.tile([H, 1], F32, tag="rmin")
        nc.vector.tensor_reduce(rmin[:], e_sb[:], axis=mybir.AxisListType.X,
                                op=mybir.AluOpType.min)
        nc.vector.tensor_scalar_mul(pack[:, 1:2], rmin[:], -1.0)
        packr = spool.tile([H, 2], F32, tag="packr")
        nc.gpsimd.partition_all_reduce(packr[:], pack[:], channels=H,
                                       reduce_op=bass_isa.ReduceOp.max)

        cbuf = spool.tile([1, 8], F32, tag="cbuf")
        nc.vector.memset(cbuf[:], -3.0e38)
        nc.gpsimd.tensor_copy(cbuf[:, 0:2], packr[0:1, :])
        cc_in = dram.tile([1, 8], F32, tag="cc_in")
        cc_out = dram.tile([ncores, 8], F32, tag="cc_out")
        nc.scalar.dma_start(cc_in[:], cbuf[:])
        nc.gpsimd.collective_compute(
            "AllGather", mybir.AluOpType.bypass,
            replica_groups=[list(range(ncores))],
            ins=[cc_in.opt()], outs=[cc_out.opt()],
        )
        g8 = spool.tile([ncores, 2], F32, tag="g8")
        nc.scalar.dma_start(g8[:], cc_out[:, 0:2])
        g8r = spool.tile([ncores, 2], F32, tag="g8r")
        nc.gpsimd.partition_all_reduce(g8r[:], g8[:], channels=ncores,
                                       reduce_op=bass_isa.ReduceOp.max)

        # PE warm-up burst #2: gated on the AllGather result landing (g8) via
        # a DVE copy feeding a K=1 matmul, bypassing the busy gpsimd queue so
        # it starts right when the collective returns. The remaining burst
        # matmuls are WAW-chained on ps_w so Tile cannot hoist them earlier.
        g2b = spool.tile([1, 1], BF16, tag="g2b")
        nc.vector.tensor_copy(g2b[:], g8[0:1, 0:1])
        nc.tensor.matmul(ps_w[0:1, :], g2b[:], wtile0[0:1, :],
                         start=True, stop=True)
        for _ in range(NWARM2 - 1):
            nc.tensor.matmul(ps_w[:], wtile0[:], wtile0[:], start=True, stop=True)

        # broadcast (gmax, -gmin) to all partitions; vectorized softmax prep
        g128 = spool.tile([H, 2], F32, tag="g128")
        nc.gpsimd.partition_broadcast(g128[:], g8r[0:1, :])
        rng_t = spool.tile([H, 1], F32, tag="rng")
        nc.vector.tensor_tensor(rng_t[:], g128[:, 0:1], g128[:, 1:2],
                                mybir.AluOpType.add)
        inv_t = spool.tile([H, 1], F32, tag="inv")
        nc.vector.reciprocal(inv_t[:], rng_t[:])
        bias_t = spool.tile([H, 1], F32, tag="bias")
        nc.vector.tensor_tensor(bias_t[:], g128[:, 1:2], inv_t[:],
                                mybir.AluOpType.mult)

        s_sb = spool.tile([H, 2 * KH], F32, tag="s_sb")
        ssum = spool.tile([H, 1], F32, tag="ssum")
        nc.scalar.activation(s_sb[:], e_sb[:], mybir.ActivationFunctionType.Exp,
                             bias=bias_t[:], scale=inv_t[:], accum_out=ssum[:])
        stot = spool.tile([H, 1], F32, tag="stot")
        nc.gpsimd.partition_all_reduce(stot[:], ssum[:], channels=H,
                                       reduce_op=bass_isa.ReduceOp.add)
        rn = spool.tile([H, 1], F32, tag="rn")
        nc.vector.reciprocal(rn[:], stot[:])
        s16 = spool.tile([H, 2 * KH], BF16, tag="s16")
        nc.vector.tensor_scalar_mul(s16[:], s_sb[:], rn[:])

        # att transposes + A-mat builds
        apool = top.enter_context(tc.tile_pool(name="amats", bufs=1))
        with tc.tile_pool(name="ps_a", bufs=1, space="PSUM") as ps_a:
            ps_tH = ps_a.tile([KH, H], BF16, tag="ps_tH")
            nc.tensor.transpose(ps_tH[:], s16[:, 0:KH], c_ident[:])
            att_kh = spool.tile([KH, H], BF16, tag="att_kh")
            nc.scalar.copy(att_kh[:], ps_tH[:])
            ps_tW = ps_a.tile([KH, W], BF16, tag="ps_tW")
            nc.tensor.transpose(ps_tW[:], s16[:, KH:2 * KH], c_ident[:])
            att_kw = spool.tile([KH, W], BF16, tag="att_kw")
            nc.scalar.copy(att_kw[:], ps_tW[:])

            # A4[32r+k, h] = 0.5 * att_H[h, k] on the 4 strips
            ps_a4 = ps_a.tile([128, H], F32, tag="ps_a4")
            nc.tensor.matmul(ps_a4[:], c_rep4[:], att_kh[:], start=True, stop=True)
            A4 = apool.tile([128, H], BF16, tag="A4")
            nc.vector.tensor_copy(A4[:], ps_a4[:])

            # M_WI = 0.0625 * att_W expanded + I (residual rides the I)
            ps_mw = ps_a.tile([W, W], F32, tag="ps_mw")
            nc.tensor.matmul(ps_mw[:], c_expmat[:], att_kw[:], start=True, stop=True)
            M_WI = apool.tile([W, W], BF16, tag="M_WI")
            nc.vector.scalar_tensor_tensor(M_WI[:], ps_mw[:], 1.0, c_eye[:],
                                           op0=mybir.AluOpType.mult,
                                           op1=mybir.AluOpType.add)

        psb.close()

        # ================= phase 2 =================
        with ExitStack() as ph2:
            ps_out = ph2.enter_context(tc.tile_pool(name="ps_out", bufs=2, space="PSUM"))
            opool = ph2.enter_context(tc.tile_pool(name="outp", bufs=3))
            for ch in range(nchunks):
                c0 = ch * CHUNK
                t2tg = t2t_chunks[ch]
                ob = opool.tile([H, CHUNK * W], F16, tag="ob")
                for s in range(CHUNK // G):
                    qoff = s * G * W
                    toff = s * G * H
                    vbase = ch * CHUNK * W + qoff
                    ps_o = ps_out.tile([H, G * W], F32, tag="ps_o")
                    # out_H: 4 concurrent row-tiled K=16 matmuls from val4
                    for r in range(4):
                        nc.tensor.matmul(ps_o[:, 512 * r:512 * (r + 1)],
                                         A4[32 * r:32 * r + KH, :],
                                         val4[32 * r:32 * r + KH,
                                              vbase + 512 * r:vbase + 512 * (r + 1)],
                                         start=True, stop=False,
                                         tile_position=(32 * r, 0))
                    # out_W + residual: fp16 stationary per channel
                    for i in range(G):
                        nc.tensor.matmul(ps_o[:, i * W:(i + 1) * W],
                                         t2tg[:, toff + i * H:toff + (i + 1) * H],
                                         M_WI[:], start=False, stop=(i % 4 == 3))
                    half = G * W // 2
                    nc.scalar.copy(ob[:, qoff:qoff + half], ps_o[:, 0:half])
                    nc.vector.tensor_copy(ob[:, qoff + half:qoff + G * W],
                                          ps_o[:, half:])
                if ch == nchunks - 1:
                    # split the final transfer so the tail DMA is half-size
                    for s in range(CHUNK // G):
                        qo = s * G * W
                        nc.sync.dma_start(
                            out[:, c0 + s * G:c0 + (s + 1) * G, :],
                            ob[:, qo:qo + G * W].rearrange(
                                "p (c w) -> p c w", c=G))
                else:
                    nc.sync.dma_start(out[:, c0:c0 + CHUNK, :],
                                      ob[:].rearrange("p (c w) -> p c w", c=CHUNK))

    nc.compile()
    return nc


_NC_CACHE = {}


def _get_nc():
    key = (C, NCORES)
    if key not in _NC_CACHE:
        _NC_CACHE[key] = build(C, NCORES)
    return _NC_CACHE[key]


def _stage(tensor1, tensor2):
    """Host-side precision/layout staging for all cores."""
    t1b = np.ascontiguousarray(
        tensor1.astype(F8_NP).transpose(0, 2, 1, 3))            # (B,H,C,W) fp8
    t2h = np.ascontiguousarray(
        tensor2.astype(F8_NP).transpose(0, 2, 1, 3))            # (B,H,C,W) fp8
    t2t = np.ascontiguousarray(
        tensor2.astype(np.float16).transpose(0, 3, 1, 2))       # (B,W,C,H) fp16
    return t1b, t2h, t2t


def kernel(tensor1: np.ndarray, tensor2: np.ndarray) -> np.ndarray:
    from concourse.bass_utils import run_bass_kernel_spmd
    assert tensor1.shape == (B, C, H, W) and tensor2.shape == (B, C, H, W)
    nc = _get_nc()
    consts = host_constants()
    t1b, t2h, t2t = _stage(np.asarray(tensor1, np.float32),
                           np.asarray(tensor2, np.float32))
    in_maps = [
        {"t1b": t1b[b], "t2h": t2h[b], "t2t": t2t[b], **consts}
        for b in range(B)
    ]
    res = run_bass_kernel_spmd(nc, in_maps, core_ids=list(range(NCORES)))
    out_hcw = np.stack([res.results[b]["out"] for b in range(B)])  # (B,H,C,W) f16
    return np.ascontiguousarray(
        out_hcw.transpose(0, 2, 1, 3).astype(np.float32))

